# revision 31
# baseline (speedup 1.0000x reference)
"""NeuralODE RK4 kernel for TRN2 (8 NeuronCores, data-parallel over batch).

Reference computation (per step, h=0.05, 20 steps):
  f(z,t) = tanh(tanh([z,t] @ W1 + b1) @ W2 + b2) @ W3 + b3
  RK4: k1..k4, z += h/6 (k1 + 2k2 + 2k3 + k4)

Device layout ("dataflow A"): every activation lives transposed
[feature partitions, batch free], weights stay in natural [in,out] chunk
layout and act as matmul stationary (lhsT).  out = W_chunk.T @ act_chunk
emerges transposed, feeding the next layer with zero transposes.

Per core: batch slice of 128 columns.  SBUF layouts (partition dim 128):
  z_sb   [128, 8*128]  fp32   z_sb[p, c*128+b]    = z[b, c*128+p]
  zbf    [128, 8*128]  bf16   (matmul input copy of z)
  h1bf   [128, 16*128] bf16   h1bf[p, j*128+b]    = h1[b, j*128+p]
  h2bf   [128, 16*128] bf16
  k_sb   [128, 8*128]  fp32   current RK4 stage k
  acc    [128, 8*128]  fp32   running z + h/6*(k1+2k2+..)
  w1_sb  [128, 8*16*128]  bf16  w1_sb[kk, (k*16+j)*128+m] = W1[k*128+kk, j*128+m]
  w2_sb  [128, 16*16*128] bf16
  w3_sb  [128, 16*8*128]  bf16
  b1eff  [128, 80*16] fp32   b1eff[p, s*16+j] = b1[j*128+p] + t_s*W1[1024, j*128+p]
  b2_sb  [128, 16], b3_sb [128, 8] fp32
"""

import os

import numpy as np
import ml_dtypes

NUM_STEPS = 20
H = 1.0 / NUM_STEPS
B, D, HID = 1024, 1024, 2048
NCORES = 8
BC = B // NCORES  # 128 batch per core

DK = D // 128      # 8   dim chunks
HK = HID // 128    # 16  hidden chunks

LAST_EXEC_NS = None

_CACHE = {}


def _build_bass():
    import concourse.bass as bass
    import concourse.mybir as mybir
    from concourse.tile import TileContext

    f32 = mybir.dt.float32
    bf16 = mybir.dt.bfloat16
    ALU = mybir.AluOpType
    ACTF = mybir.ActivationFunctionType

    nc = bass.Bass()

    # weights concatenated [w1|w2|w3] along free dim; biases [b1eff|b2|b3].
    # One DMA each: every DMA instruction claims a DMAHW sem lane, and the
    # loop-reset Drain can only wait on ~8 sems total (3 engine + DMA lanes).
    WN = (DK * HK + HK * HK + HK * DK) * 128  # 69632
    W1OFF = 0
    W2OFF = DK * HK * 128          # 16384
    W3OFF = W2OFF + HK * HK * 128  # 49152
    BN = 4 * NUM_STEPS * HK + HK + DK  # 1304
    B2OFF = 4 * NUM_STEPS * HK     # 1280
    B3OFF = B2OFF + HK             # 1296

    z_in = nc.declare_dram_parameter("z_in", [128, DK * 128], f32, isOutput=False)
    w_in = nc.declare_dram_parameter("w_in", [128, WN], bf16, isOutput=False)
    b_in = nc.declare_dram_parameter("b_in", [128, BN], f32, isOutput=False)
    z_out = nc.declare_dram_parameter("z_out", [128, DK * 128], f32, isOutput=True)

    with TileContext(nc) as tc:
        with (
            tc.tile_pool(name="persist", bufs=1) as pp,
            tc.tile_pool(name="psum", bufs=7, space="PSUM") as psp,
            tc.tile_pool(name="psumscr", bufs=1, space="PSUM") as psc,
        ):
            w_all = pp.tile([128, WN], bf16)
            b_all = pp.tile([128, BN], f32)
            z_sb = pp.tile([128, DK * 128], f32)
            zbf = pp.tile([128, DK * 128], bf16)
            h1bf = pp.tile([128, HK * 128], bf16)
            h2bf = pp.tile([128, HK * 128], bf16)
            k_sb = pp.tile([128, DK * 128], f32)
            acc = pp.tile([128, DK * 128], f32)

            nc.sync.dma_start(out=w_all[:], in_=w_in[:])
            nc.scalar.dma_start(out=b_all[:], in_=b_in[:])
            nc.sync.dma_start(out=z_sb[:], in_=z_in[:])

            # Dummy SP-engine reads of the LAST elements of each DMA'd tile:
            # SP absorbs the full DMAHW completion waits here, so the final
            # Drain (also on SP) elides them.  walrus on this image only
            # supports ONE sync wait per instruction, so the end-of-context
            # Drain must be left with at most the output-DMA wait.
            i32 = mybir.dt.int32
            with nc.sync.register() as r:
                nc.sync.reg_load(r, w_all[127:128, WN - 2:WN].bitcast(i32))
                nc.sync.reg_load(r, b_all[127:128, BN - 2:BN].bitcast(i32))
                nc.sync.reg_load(r, z_sb[127:128, DK * 128 - 2:DK * 128].bitcast(i32))

            nc.scalar.activation(out=zbf[:], in_=z_sb[:], func=ACTF.Copy)

            # Wait-host dummies: walrus allows only ~1 sync wait per compute
            # instruction, and the tile framework can only host extra waits
            # on earlier SAME-engine instructions.  These tiny ops give ACT/
            # DVE a place to absorb the DMA-lane waits up front.
            scr = pp.tile([128, 8], f32)
            scr_d = pp.tile([128, 2], f32)
            nc.scalar.activation(out=scr[:, 0:2], in_=w_all[:, WN - 2:WN], func=ACTF.Copy)
            nc.scalar.activation(out=scr[:, 2:4], in_=b_all[:, BN - 2:BN], func=ACTF.Copy)
            nc.vector.tensor_scalar(
                scr_d[:], z_sb[:, DK * 128 - 2:DK * 128], 1.0, scalar2=None, op0=ALU.mult,
            )

            # RK4 stage coefficients
            ZIN_C = [0.5 * H, 0.5 * H, H]          # z_in for stages 2,3,4
            ACC_C = [H / 6.0, H / 3.0, H / 3.0, H / 6.0]

            # Fully unrolled: this walrus supports only ONE sync wait per
            # instruction, which is incompatible with For_i's reset-block
            # Drain and exit all-engine-barrier NoOps.  Straight-line code
            # only ever needs direct producer waits (<=1 each).
            last_ps = None
            for ev in range(NUM_STEPS * 4):
                st = ev % 4
                if ev > 0:
                    # Per-eval wait hosts: absorb "previous eval's DVE work
                    # done" (zbf fully rewritten) into ACT and PE program
                    # order so the real instructions below carry <=1 wait.
                    zlast = zbf[:, DK * 128 - 2:DK * 128]
                    nc.scalar.activation(out=scr[:, 4:6], in_=zlast, func=ACTF.Copy)
                    ps_scr = psc.tile([128, 2], f32, tag="pscr")
                    nc.tensor.matmul(
                        ps_scr[0:2, 0:2], w_all[0:1, 0:2], zbf[0:1, DK * 128 - 2:DK * 128],
                        start=True, stop=True,
                    )
                # ---- layer 1: h1 = tanh(W1.T @ z + b1eff[ev]) ----
                for j in range(HK):
                    ps1 = psp.tile([128, BC], f32, tag="ps")
                    for k in range(DK):
                        nc.tensor.matmul(
                            ps1[:],
                            w_all[:, W1OFF + (k * HK + j) * 128:W1OFF + (k * HK + j) * 128 + 128],
                            zbf[:, k * 128:(k + 1) * 128],
                            start=(k == 0),
                            stop=(k == DK - 1),
                        )
                    nc.scalar.activation(
                        out=h1bf[:, j * 128:(j + 1) * 128],
                        in_=ps1[:],
                        func=ACTF.Tanh,
                        bias=b_all[:, ev * HK + j:ev * HK + j + 1],
                    )
                # ---- layer 2: h2 = tanh(W2.T @ h1 + b2) ----
                for j in range(HK):
                    ps2 = psp.tile([128, BC], f32, tag="ps")
                    for k in range(HK):
                        nc.tensor.matmul(
                            ps2[:],
                            w_all[:, W2OFF + (k * HK + j) * 128:W2OFF + (k * HK + j) * 128 + 128],
                            h1bf[:, k * 128:(k + 1) * 128],
                            start=(k == 0),
                            stop=(k == HK - 1),
                        )
                    nc.scalar.activation(
                        out=h2bf[:, j * 128:(j + 1) * 128],
                        in_=ps2[:],
                        func=ACTF.Tanh,
                        bias=b_all[:, B2OFF + j:B2OFF + j + 1],
                    )
                # ---- layer 3 + RK4 update per output chunk ----
                for c in range(DK):
                    ps3 = psp.tile([128, BC], f32, tag="ps")
                    for k in range(HK):
                        nc.tensor.matmul(
                            ps3[:],
                            w_all[:, W3OFF + (k * DK + c) * 128:W3OFF + (k * DK + c) * 128 + 128],
                            h2bf[:, k * 128:(k + 1) * 128],
                            start=(k == 0),
                            stop=(k == HK - 1),
                        )
                    last_ps = ps3
                    cs = slice(c * 128, (c + 1) * 128)
                    nc.scalar.activation(
                        out=k_sb[:, cs],
                        in_=ps3[:],
                        func=ACTF.Identity,
                        bias=b_all[:, B3OFF + c:B3OFF + c + 1],
                    )
                    if st == 0:
                        nc.vector.scalar_tensor_tensor(
                            out=acc[:, cs], in0=k_sb[:, cs], scalar=ACC_C[0],
                            in1=z_sb[:, cs], op0=ALU.mult, op1=ALU.add,
                        )
                    elif st < 3:
                        nc.vector.scalar_tensor_tensor(
                            out=acc[:, cs], in0=k_sb[:, cs], scalar=ACC_C[st],
                            in1=acc[:, cs], op0=ALU.mult, op1=ALU.add,
                        )
                    if st < 3:
                        nc.vector.scalar_tensor_tensor(
                            out=zbf[:, cs], in0=k_sb[:, cs], scalar=ZIN_C[st],
                            in1=z_sb[:, cs], op0=ALU.mult, op1=ALU.add,
                        )
                    else:
                        nc.vector.scalar_tensor_tensor(
                            out=z_sb[:, cs], in0=k_sb[:, cs], scalar=ACC_C[3],
                            in1=acc[:, cs], op0=ALU.mult, op1=ALU.add,
                        )
                        if ev != NUM_STEPS * 4 - 1:
                            nc.vector.scalar_tensor_tensor(
                                out=zbf[:, cs], in0=k_sb[:, cs], scalar=ACC_C[3],
                                in1=acc[:, cs], op0=ALU.mult, op1=ALU.add,
                            )

            # Absorb ACT/DVE final semaphore values into SP program order so
            # the end-of-context Drain (on SP) elides them -- walrus here
            # supports only ONE sync wait per Drain.
            with nc.sync.register() as r2:
                nc.sync.reg_load(r2, k_sb[127:128, DK * 128 - 2:DK * 128].bitcast(i32))
                nc.sync.reg_load(r2, z_sb[127:128, DK * 128 - 2:DK * 128].bitcast(i32))

            nc.sync.dma_start(out=z_out[:], in_=z_sb[:])

    # walrus on this image allows very few sync waits per instruction, so
    # trim provably-redundant waits:
    #  (a) same-engine self-waits (engines execute their compute queue in
    #      order, so an instruction never needs to wait on its own engine's
    #      tile semaphore), and
    #  (b) the final SP Drain's PE wait: SP cannot observe PE directly
    #      (PSUM reads are untracked), but the drain's remaining store-DMA
    #      wait transitively implies PE quiesced (store waits DVE>=final,
    #      DVE waited ACT>=final, ACT waited PE>=final).
    eng_pref = {"Activation": "Activation_", "PE": "PE_", "DVE": "DVE_", "SP": "SP_", "Pool": "Pool_"}
    for inst in nc.inst_map.values():
        si = getattr(inst, "sync_info", None)
        if si is None or not si.on_wait:
            continue
        pref = eng_pref.get(getattr(getattr(inst, "engine", None), "name", None))
        if pref:
            kept = [w for w in si.on_wait if not str(w.ant_name).startswith(pref)]
            if len(kept) != len(si.on_wait):
                si.on_wait = kept
    # Host excess Matmult waits on the immediately-preceding Ldweights of the
    # SAME matmul (PE queue: LW then MM back-to-back, LW never waits and does
    # not tick PE_44).  Waiting at the LW still strictly precedes the PSUM
    # write, and the awaited ACT/DVE producers never depend on this LW, so no
    # deadlock is possible.  This is needed because the scheduler re-orders
    # our per-eval dummy MMs *after* the first real MM of the eval.
    import bass_rust
    for bbw in nc.bb_map.values():
        prev_pe = None
        for binst in bbw.bb.instructions:
            inst = nc.inst_map.get(binst.name, binst)
            if getattr(getattr(inst, "engine", None), "name", None) != "PE":
                continue
            si = getattr(inst, "sync_info", None)
            if si is not None and si.on_wait and len(si.on_wait) > 1:
                if (
                    prev_pe is not None
                    and type(prev_pe).__name__ == "InstLdweights"
                    and getattr(prev_pe, "sync_info", None) is None
                ):
                    waits = list(si.on_wait)
                    prev_pe.sync_info = bass_rust.SyncInfo(
                        on_wait=[waits[0]], on_update=[]
                    )
                    si.on_wait = waits[1:]
            prev_pe = inst
    # Generic monotone elision the framework missed: within a block, if an
    # earlier instruction on the SAME engine already waited sem >= V, any
    # later wait sem >= v with v <= V is redundant (tile sems only tick up
    # and there are no loops/resets in this straight-line kernel).
    for bbw in nc.bb_map.values():
        observed = {}
        for binst in bbw.bb.instructions:
            inst = nc.inst_map.get(binst.name, binst)
            si = getattr(inst, "sync_info", None)
            if si is None or not si.on_wait:
                continue
            eng = getattr(getattr(inst, "engine", None), "name", None)
            obs = observed.setdefault(eng, {})
            kept = []
            for w in si.on_wait:
                nm = str(w.ant_name)
                if w.wait_mode == "sem-ge-imm" and ("_4" in nm or nm.startswith("DMA")):
                    if obs.get(nm, -1) >= w.wait_value:
                        continue
                    obs[nm] = w.wait_value
                kept.append(w)
            if len(kept) != len(si.on_wait):
                si.on_wait = kept
    trimmed = 0
    for inst in nc.inst_map.values():
        si = getattr(inst, "sync_info", None)
        if si is None or not si.on_wait:
            continue
        if type(inst).__name__ == "InstDrain" and len(si.on_wait) > 1:
            keep = [w for w in si.on_wait if str(w.ant_name).startswith("DMAHW")]
            assert len(keep) == 1, [str(w) for w in si.on_wait]
            si.on_wait = keep
            trimmed += 1
    assert trimmed == 1, trimmed

    return nc


def _prep_inputs(z0, W1, b1, W2, b2, W3, b3):
    bf = ml_dtypes.bfloat16
    W1p = np.ascontiguousarray(W1[:D])
    w1_host = np.ascontiguousarray(
        W1p.reshape(DK, 128, HK, 128).transpose(1, 0, 2, 3).reshape(128, DK * HK * 128)
    ).astype(bf)
    w2_host = np.ascontiguousarray(
        W2.reshape(HK, 128, HK, 128).transpose(1, 0, 2, 3).reshape(128, HK * HK * 128)
    ).astype(bf)
    w3_host = np.ascontiguousarray(
        W3.reshape(HK, 128, DK, 128).transpose(1, 0, 2, 3).reshape(128, HK * DK * 128)
    ).astype(bf)

    t_steps = H * np.arange(NUM_STEPS, dtype=np.float64)
    stage_t = np.stack(
        [t_steps, t_steps + 0.5 * H, t_steps + 0.5 * H, t_steps + H], axis=1
    ).reshape(-1)  # [80]
    b1eff = b1[None, :].astype(np.float64) + stage_t[:, None] * W1[D][None, :].astype(np.float64)
    b1eff_host = np.ascontiguousarray(
        b1eff.reshape(4 * NUM_STEPS, HK, 128).transpose(2, 0, 1).reshape(128, 4 * NUM_STEPS * HK)
    ).astype(np.float32)

    b2_host = np.ascontiguousarray(b2.reshape(HK, 128).T).astype(np.float32)
    b3_host = np.ascontiguousarray(b3.reshape(DK, 128).T).astype(np.float32)

    w_host = np.ascontiguousarray(np.concatenate([w1_host, w2_host, w3_host], axis=1))
    b_host = np.ascontiguousarray(
        np.concatenate([b1eff_host, b2_host, b3_host], axis=1)
    )
    shared = {"w_in": w_host, "b_in": b_host}
    in_maps = []
    for n in range(NCORES):
        zc = z0[n * BC:(n + 1) * BC]  # [128 b, 1024 d]
        z_host = np.ascontiguousarray(
            zc.T.reshape(DK, 128, BC).transpose(1, 0, 2).reshape(128, DK * BC)
        ).astype(np.float32)
        in_maps.append({"z_in": z_host, **shared})
    return in_maps


def kernel(z0, W1, b1, W2, b2, W3, b3):
    global LAST_EXEC_NS
    from concourse.bass_utils import run_bass_kernel_spmd

    z0 = np.asarray(z0, dtype=np.float32)
    W1 = np.asarray(W1, dtype=np.float32)
    b1 = np.asarray(b1, dtype=np.float32)
    W2 = np.asarray(W2, dtype=np.float32)
    b2 = np.asarray(b2, dtype=np.float32)
    W3 = np.asarray(W3, dtype=np.float32)
    b3 = np.asarray(b3, dtype=np.float32)

    if "nc" not in _CACHE:
        _CACHE["nc"] = _build_bass()
    nc = _CACHE["nc"]

    in_maps = _prep_inputs(z0, W1, b1, W2, b2, W3, b3)

    trace = bool(int(os.environ.get("NEURALODE_TRACE", "0")))
    res = run_bass_kernel_spmd(nc, in_maps, core_ids=list(range(NCORES)), trace=trace)
    LAST_EXEC_NS = res.exec_time_ns

    out = np.empty((B, D), dtype=np.float32)
    for n in range(NCORES):
        o = np.asarray(res.results[n]["z_out"], dtype=np.float32)  # [128, 1024]
        zc = o.reshape(128, DK, BC).transpose(1, 0, 2).reshape(D, BC).T  # [128 b, 1024 d]
        out[n * BC:(n + 1) * BC] = zc
    return out


# revision 35
# speedup vs baseline: 1.0010x; 1.0010x over previous
"""NeuralODE RK4 kernel for TRN2 (8 NeuronCores, data-parallel over batch).

Reference computation (per step, h=0.05, 20 steps):
  f(z,t) = tanh(tanh([z,t] @ W1 + b1) @ W2 + b2) @ W3 + b3
  RK4: k1..k4, z += h/6 (k1 + 2k2 + 2k3 + k4)

Device layout ("dataflow A"): every activation lives transposed
[feature partitions, batch free], weights stay in natural [in,out] chunk
layout and act as matmul stationary (lhsT).  out = W_chunk.T @ act_chunk
emerges transposed, feeding the next layer with zero transposes.

Per core: batch slice of 128 columns.  SBUF layouts (partition dim 128):
  z_sb   [128, 8*128]  fp32   z_sb[p, c*128+b]    = z[b, c*128+p]
  zbf    [128, 8*128]  bf16   (matmul input copy of z)
  h1bf   [128, 16*128] bf16   h1bf[p, j*128+b]    = h1[b, j*128+p]
  h2bf   [128, 16*128] bf16
  k_sb   [128, 8*128]  fp32   current RK4 stage k
  acc    [128, 8*128]  fp32   running z + h/6*(k1+2k2+..)
  w1_sb  [128, 8*16*128]  bf16  w1_sb[kk, (k*16+j)*128+m] = W1[k*128+kk, j*128+m]
  w2_sb  [128, 16*16*128] bf16
  w3_sb  [128, 16*8*128]  bf16
  b1eff  [128, 80*16] fp32   b1eff[p, s*16+j] = b1[j*128+p] + t_s*W1[1024, j*128+p]
  b2_sb  [128, 16], b3_sb [128, 8] fp32
"""

import os

import numpy as np
import ml_dtypes

NUM_STEPS = 20
H = 1.0 / NUM_STEPS
B, D, HID = 1024, 1024, 2048
NCORES = 8
BC = B // NCORES  # 128 batch per core

DK = D // 128      # 8   dim chunks
HK = HID // 128    # 16  hidden chunks

LAST_EXEC_NS = None

_CACHE = {}


def _build_bass():
    import concourse.bass as bass
    import concourse.mybir as mybir
    from concourse.tile import TileContext

    f32 = mybir.dt.float32
    bf16 = mybir.dt.bfloat16
    ALU = mybir.AluOpType
    ACTF = mybir.ActivationFunctionType

    nc = bass.Bass()

    # weights concatenated [w1|w2|w3] along free dim; biases [b1eff|b2|b3].
    # One DMA each: every DMA instruction claims a DMAHW sem lane, and the
    # loop-reset Drain can only wait on ~8 sems total (3 engine + DMA lanes).
    WN = (DK * HK + HK * HK + HK * DK) * 128  # 69632
    W1OFF = 0
    W2OFF = DK * HK * 128          # 16384
    W3OFF = W2OFF + HK * HK * 128  # 49152
    BN = 4 * NUM_STEPS * HK + HK + DK  # 1304
    B2OFF = 4 * NUM_STEPS * HK     # 1280
    B3OFF = B2OFF + HK             # 1296

    z_in = nc.declare_dram_parameter("z_in", [128, DK * 128], f32, isOutput=False)
    w_in = nc.declare_dram_parameter("w_in", [128, WN], bf16, isOutput=False)
    b_in = nc.declare_dram_parameter("b_in", [128, BN], f32, isOutput=False)
    z_out = nc.declare_dram_parameter("z_out", [128, DK * 128], f32, isOutput=True)

    with TileContext(nc) as tc:
        with (
            tc.tile_pool(name="persist", bufs=1) as pp,
            tc.tile_pool(name="psum", bufs=8, space="PSUM") as psp,
        ):
            w_all = pp.tile([128, WN], bf16)
            b_all = pp.tile([128, BN], f32)
            z_sb = pp.tile([128, DK * 128], f32)
            zbf = pp.tile([128, DK * 128], bf16)
            h1bf = pp.tile([128, HK * 128], bf16)
            h2bf = pp.tile([128, HK * 128], bf16)
            k_sb = pp.tile([128, DK * 128], f32)
            acc = pp.tile([128, DK * 128], f32)

            # Per-layer weight DMAs: L1 matmuls only need W1, so compute can
            # start as soon as the first ~4MB lands instead of all 17.8MB.
            nc.sync.dma_start(out=w_all[:, W1OFF:W2OFF], in_=w_in[:, W1OFF:W2OFF])
            nc.sync.dma_start(out=w_all[:, W2OFF:W3OFF], in_=w_in[:, W2OFF:W3OFF])
            nc.sync.dma_start(out=w_all[:, W3OFF:WN], in_=w_in[:, W3OFF:WN])
            nc.scalar.dma_start(out=b_all[:], in_=b_in[:])
            nc.sync.dma_start(out=z_sb[:], in_=z_in[:])

            # Dummy SP-engine reads of the LAST elements of each DMA'd tile:
            # SP absorbs the full DMAHW completion waits here, so the final
            # Drain (also on SP) elides them.  walrus on this image only
            # supports ONE sync wait per instruction, so the end-of-context
            # Drain must be left with at most the output-DMA wait.
            i32 = mybir.dt.int32
            with nc.sync.register() as r:
                nc.sync.reg_load(r, w_all[127:128, W2OFF - 2:W2OFF].bitcast(i32))
                nc.sync.reg_load(r, w_all[127:128, W3OFF - 2:W3OFF].bitcast(i32))
                nc.sync.reg_load(r, w_all[127:128, WN - 2:WN].bitcast(i32))
                nc.sync.reg_load(r, b_all[127:128, BN - 2:BN].bitcast(i32))
                nc.sync.reg_load(r, z_sb[127:128, DK * 128 - 2:DK * 128].bitcast(i32))

            nc.scalar.activation(out=zbf[:], in_=z_sb[:], func=ACTF.Copy)

            # Wait-host dummies: walrus allows only ~1 sync wait per compute
            # instruction, and the tile framework can only host extra waits
            # on earlier SAME-engine instructions.  These tiny ops give ACT/
            # DVE a place to absorb the DMA-lane waits up front.
            scr = pp.tile([128, 8], f32)
            scr_d = pp.tile([128, 2], f32)
            nc.scalar.activation(out=scr[:, 0:2], in_=w_all[:, WN - 2:WN], func=ACTF.Copy)
            nc.scalar.activation(out=scr[:, 2:4], in_=b_all[:, BN - 2:BN], func=ACTF.Copy)
            nc.vector.tensor_scalar(
                scr_d[:], z_sb[:, DK * 128 - 2:DK * 128], 1.0, scalar2=None, op0=ALU.mult,
            )

            # RK4 stage coefficients
            ZIN_C = [0.5 * H, 0.5 * H, H]          # z_in for stages 2,3,4
            ACC_C = [H / 6.0, H / 3.0, H / 3.0, H / 6.0]

            # Fully unrolled: this walrus supports only ONE sync wait per
            # instruction, which is incompatible with For_i's reset-block
            # Drain and exit all-engine-barrier NoOps.  Straight-line code
            # only ever needs direct producer waits (<=1 each).
            last_ps = None
            for ev in range(NUM_STEPS * 4):
                st = ev % 4
                if ev > 0:
                    # Per-eval ACT wait host: absorb "previous eval's DVE
                    # work done" (zbf fully rewritten) into ACT program
                    # order so the real activations below carry <=1 wait.
                    # (PE excess waits are hosted on Ldweights post-hoc.)
                    zlast = zbf[:, DK * 128 - 2:DK * 128]
                    nc.scalar.activation(out=scr[:, 4:6], in_=zlast, func=ACTF.Copy)
                # ---- layer 1: h1 = tanh(W1.T @ z + b1eff[ev]) ----
                for j in range(HK):
                    ps1 = psp.tile([128, BC], f32, tag="ps")
                    for k in range(DK):
                        nc.tensor.matmul(
                            ps1[:],
                            w_all[:, W1OFF + (k * HK + j) * 128:W1OFF + (k * HK + j) * 128 + 128],
                            zbf[:, k * 128:(k + 1) * 128],
                            start=(k == 0),
                            stop=(k == DK - 1),
                        )
                    nc.scalar.activation(
                        out=h1bf[:, j * 128:(j + 1) * 128],
                        in_=ps1[:],
                        func=ACTF.Tanh,
                        bias=b_all[:, ev * HK + j:ev * HK + j + 1],
                    )
                # ---- layer 2: h2 = tanh(W2.T @ h1 + b2) ----
                for j in range(HK):
                    ps2 = psp.tile([128, BC], f32, tag="ps")
                    for k in range(HK):
                        nc.tensor.matmul(
                            ps2[:],
                            w_all[:, W2OFF + (k * HK + j) * 128:W2OFF + (k * HK + j) * 128 + 128],
                            h1bf[:, k * 128:(k + 1) * 128],
                            start=(k == 0),
                            stop=(k == HK - 1),
                        )
                    nc.scalar.activation(
                        out=h2bf[:, j * 128:(j + 1) * 128],
                        in_=ps2[:],
                        func=ACTF.Tanh,
                        bias=b_all[:, B2OFF + j:B2OFF + j + 1],
                    )
                # ---- layer 3 + RK4 update per output chunk ----
                for c in range(DK):
                    ps3 = psp.tile([128, BC], f32, tag="ps")
                    for k in range(HK):
                        nc.tensor.matmul(
                            ps3[:],
                            w_all[:, W3OFF + (k * DK + c) * 128:W3OFF + (k * DK + c) * 128 + 128],
                            h2bf[:, k * 128:(k + 1) * 128],
                            start=(k == 0),
                            stop=(k == HK - 1),
                        )
                    last_ps = ps3
                    cs = slice(c * 128, (c + 1) * 128)
                    nc.scalar.activation(
                        out=k_sb[:, cs],
                        in_=ps3[:],
                        func=ACTF.Identity,
                        bias=b_all[:, B3OFF + c:B3OFF + c + 1],
                    )
                    if st == 0:
                        nc.vector.scalar_tensor_tensor(
                            out=acc[:, cs], in0=k_sb[:, cs], scalar=ACC_C[0],
                            in1=z_sb[:, cs], op0=ALU.mult, op1=ALU.add,
                        )
                    elif st < 3:
                        nc.vector.scalar_tensor_tensor(
                            out=acc[:, cs], in0=k_sb[:, cs], scalar=ACC_C[st],
                            in1=acc[:, cs], op0=ALU.mult, op1=ALU.add,
                        )
                    if st < 3:
                        nc.vector.scalar_tensor_tensor(
                            out=zbf[:, cs], in0=k_sb[:, cs], scalar=ZIN_C[st],
                            in1=z_sb[:, cs], op0=ALU.mult, op1=ALU.add,
                        )
                    else:
                        nc.vector.scalar_tensor_tensor(
                            out=z_sb[:, cs], in0=k_sb[:, cs], scalar=ACC_C[3],
                            in1=acc[:, cs], op0=ALU.mult, op1=ALU.add,
                        )
                        if ev != NUM_STEPS * 4 - 1:
                            nc.vector.scalar_tensor_tensor(
                                out=zbf[:, cs], in0=k_sb[:, cs], scalar=ACC_C[3],
                                in1=acc[:, cs], op0=ALU.mult, op1=ALU.add,
                            )

            # Absorb ACT/DVE final semaphore values into SP program order so
            # the end-of-context Drain (on SP) elides them -- walrus here
            # supports only ONE sync wait per Drain.
            with nc.sync.register() as r2:
                nc.sync.reg_load(r2, k_sb[127:128, DK * 128 - 2:DK * 128].bitcast(i32))
                nc.sync.reg_load(r2, z_sb[127:128, DK * 128 - 2:DK * 128].bitcast(i32))

            nc.sync.dma_start(out=z_out[:], in_=z_sb[:])

    # walrus on this image allows very few sync waits per instruction, so
    # trim provably-redundant waits:
    #  (a) same-engine self-waits (engines execute their compute queue in
    #      order, so an instruction never needs to wait on its own engine's
    #      tile semaphore), and
    #  (b) the final SP Drain's PE wait: SP cannot observe PE directly
    #      (PSUM reads are untracked), but the drain's remaining store-DMA
    #      wait transitively implies PE quiesced (store waits DVE>=final,
    #      DVE waited ACT>=final, ACT waited PE>=final).
    eng_pref = {"Activation": "Activation_", "PE": "PE_", "DVE": "DVE_", "SP": "SP_", "Pool": "Pool_"}
    for inst in nc.inst_map.values():
        si = getattr(inst, "sync_info", None)
        if si is None or not si.on_wait:
            continue
        pref = eng_pref.get(getattr(getattr(inst, "engine", None), "name", None))
        if pref:
            kept = [w for w in si.on_wait if not str(w.ant_name).startswith(pref)]
            if len(kept) != len(si.on_wait):
                si.on_wait = kept
    # Host excess Matmult waits on the immediately-preceding Ldweights of the
    # SAME matmul (PE queue: LW then MM back-to-back, LW never waits and does
    # not tick PE_44).  Waiting at the LW still strictly precedes the PSUM
    # write, and the awaited ACT/DVE producers never depend on this LW, so no
    # deadlock is possible.  This is needed because the scheduler re-orders
    # our per-eval dummy MMs *after* the first real MM of the eval.
    import bass_rust
    for bbw in nc.bb_map.values():
        prev_pe = None
        for binst in bbw.bb.instructions:
            inst = nc.inst_map.get(binst.name, binst)
            if getattr(getattr(inst, "engine", None), "name", None) != "PE":
                continue
            si = getattr(inst, "sync_info", None)
            if si is not None and si.on_wait and len(si.on_wait) > 1:
                if (
                    prev_pe is not None
                    and type(prev_pe).__name__ == "InstLdweights"
                    and getattr(prev_pe, "sync_info", None) is None
                ):
                    waits = list(si.on_wait)
                    prev_pe.sync_info = bass_rust.SyncInfo(
                        on_wait=[waits[0]], on_update=[]
                    )
                    si.on_wait = waits[1:]
            prev_pe = inst
    # Generic monotone elision the framework missed: within a block, if an
    # earlier instruction on the SAME engine already waited sem >= V, any
    # later wait sem >= v with v <= V is redundant (tile sems only tick up
    # and there are no loops/resets in this straight-line kernel).
    for bbw in nc.bb_map.values():
        observed = {}
        for binst in bbw.bb.instructions:
            inst = nc.inst_map.get(binst.name, binst)
            si = getattr(inst, "sync_info", None)
            if si is None or not si.on_wait:
                continue
            eng = getattr(getattr(inst, "engine", None), "name", None)
            obs = observed.setdefault(eng, {})
            kept = []
            for w in si.on_wait:
                nm = str(w.ant_name)
                if w.wait_mode == "sem-ge-imm" and ("_4" in nm or nm.startswith("DMA")):
                    if obs.get(nm, -1) >= w.wait_value:
                        continue
                    obs[nm] = w.wait_value
                kept.append(w)
            if len(kept) != len(si.on_wait):
                si.on_wait = kept
    trimmed = 0
    for inst in nc.inst_map.values():
        si = getattr(inst, "sync_info", None)
        if si is None or not si.on_wait:
            continue
        if type(inst).__name__ == "InstDrain" and len(si.on_wait) > 1:
            keep = [w for w in si.on_wait if str(w.ant_name).startswith("DMAHW")]
            assert len(keep) == 1, [str(w) for w in si.on_wait]
            si.on_wait = keep
            trimmed += 1
    assert trimmed == 1, trimmed

    return nc


def _prep_inputs(z0, W1, b1, W2, b2, W3, b3):
    bf = ml_dtypes.bfloat16
    W1p = np.ascontiguousarray(W1[:D])
    w1_host = np.ascontiguousarray(
        W1p.reshape(DK, 128, HK, 128).transpose(1, 0, 2, 3).reshape(128, DK * HK * 128)
    ).astype(bf)
    w2_host = np.ascontiguousarray(
        W2.reshape(HK, 128, HK, 128).transpose(1, 0, 2, 3).reshape(128, HK * HK * 128)
    ).astype(bf)
    w3_host = np.ascontiguousarray(
        W3.reshape(HK, 128, DK, 128).transpose(1, 0, 2, 3).reshape(128, HK * DK * 128)
    ).astype(bf)

    t_steps = H * np.arange(NUM_STEPS, dtype=np.float64)
    stage_t = np.stack(
        [t_steps, t_steps + 0.5 * H, t_steps + 0.5 * H, t_steps + H], axis=1
    ).reshape(-1)  # [80]
    b1eff = b1[None, :].astype(np.float64) + stage_t[:, None] * W1[D][None, :].astype(np.float64)
    b1eff_host = np.ascontiguousarray(
        b1eff.reshape(4 * NUM_STEPS, HK, 128).transpose(2, 0, 1).reshape(128, 4 * NUM_STEPS * HK)
    ).astype(np.float32)

    b2_host = np.ascontiguousarray(b2.reshape(HK, 128).T).astype(np.float32)
    b3_host = np.ascontiguousarray(b3.reshape(DK, 128).T).astype(np.float32)

    w_host = np.ascontiguousarray(np.concatenate([w1_host, w2_host, w3_host], axis=1))
    b_host = np.ascontiguousarray(
        np.concatenate([b1eff_host, b2_host, b3_host], axis=1)
    )
    shared = {"w_in": w_host, "b_in": b_host}
    in_maps = []
    for n in range(NCORES):
        zc = z0[n * BC:(n + 1) * BC]  # [128 b, 1024 d]
        z_host = np.ascontiguousarray(
            zc.T.reshape(DK, 128, BC).transpose(1, 0, 2).reshape(128, DK * BC)
        ).astype(np.float32)
        in_maps.append({"z_in": z_host, **shared})
    return in_maps


def kernel(z0, W1, b1, W2, b2, W3, b3):
    global LAST_EXEC_NS
    from concourse.bass_utils import run_bass_kernel_spmd

    z0 = np.asarray(z0, dtype=np.float32)
    W1 = np.asarray(W1, dtype=np.float32)
    b1 = np.asarray(b1, dtype=np.float32)
    W2 = np.asarray(W2, dtype=np.float32)
    b2 = np.asarray(b2, dtype=np.float32)
    W3 = np.asarray(W3, dtype=np.float32)
    b3 = np.asarray(b3, dtype=np.float32)

    if "nc" not in _CACHE:
        _CACHE["nc"] = _build_bass()
    nc = _CACHE["nc"]

    in_maps = _prep_inputs(z0, W1, b1, W2, b2, W3, b3)

    trace = bool(int(os.environ.get("NEURALODE_TRACE", "0")))
    res = run_bass_kernel_spmd(nc, in_maps, core_ids=list(range(NCORES)), trace=trace)
    LAST_EXEC_NS = res.exec_time_ns

    out = np.empty((B, D), dtype=np.float32)
    for n in range(NCORES):
        o = np.asarray(res.results[n]["z_out"], dtype=np.float32)  # [128, 1024]
        zc = o.reshape(128, DK, BC).transpose(1, 0, 2).reshape(D, BC).T  # [128 b, 1024 d]
        out[n * BC:(n + 1) * BC] = zc
    return out
